# revision 1
# baseline (speedup 1.0000x reference)
"""Trainium2 Bass kernel for nn_Discriminator (conv-highway discriminator + cosine retrieval).

Model (per reference.py):
  emb = emb_w[x]                          # [64, 128, 300]
  pred     = branch(emb, conv_w*, hw_w)   # [64, 2] log-softmax
  pred_ref = branch(emb, convr_*, hwr_w)  # only rows 0..15 are used
  values[i] = sum_j cos(pred_ref[j], pred[i]);  out = log(values / values.sum())

Sharding: 80 useful row-units (64 pred rows + 16 ref rows) are split 10 per
core: core c computes the pred branch for batch rows 8c..8c+7 and the ref
branch for rows 2c, 2c+1.  Each core returns normalized log-softmax rows
[10, 2]; the host computes the tiny cosine-sum + log normalizer (O(B) work).

On-device pipeline per core:
  - indirect-DMA gather of 10*128 embedding rows (bf16) -> PE-transpose to
    channel-major embT tiles, one per 512-token chunk (so conv only waits on
    the 4 gathers feeding its chunk)
  - conv-as-matmul (bf16): weights host-packed per filter-shift k into
    [300, 1000] matrices; accumulate over (k, E-chunk) into PSUM per
    128-feature chunk; segmented reduce_max pooling straight from PSUM;
    fused bias+ReLU on the scalar engine
  - highway (bf16): batch-stationary matmul h = pooled.T @ hw_wT accumulated
    in PSUM while conv runs; bias folded in as a ones-row matmul;
    sigmoid/relu/mix epilogue, then transpose back and linear to logits
  - log_softmax + row L2-normalize -> [10, 2] per-core output
bf16 is used for the matmul-heavy path (it is the only dtype that trips the
PE's HAM clock gate to 2.4 GHz; fp32r and fp8 run at the 1.2 GHz cold clock),
with fp32 PSUM accumulation throughout. Final relative error ~1e-6.
"""

import os
import sys

for _p in ("/opt/trn_rl_repo", "/root/.axon_site/_ro/trn_rl_repo"):
    if os.path.isdir(_p) and _p not in sys.path:
        sys.path.insert(0, _p)

import ml_dtypes
import numpy as np

import concourse.bass as bass
import concourse.mybir as mybir
import concourse.tile as tile
from concourse import bacc
from concourse.bass_utils import run_bass_kernel_spmd

# ---- problem constants (hardcoded per spec) ----
B, REF, L, V, E = 64, 16, 128, 50000, 300
FS = [3, 4, 5]
NF = [300, 300, 400]
F = 1000                      # sum(NF)
NCLS = 2
N_CORES = 8
RPC = 10                      # rows per core: 8 pred + 2 ref
TOK = RPC * L                 # 1280 tokens per core
# embT is split into one tile per token-position chunk so conv matmuls only
# depend on the 4 gathers that feed their chunk: widths (cols are chunk-local)
EBW = [516, 516, 384]         # chunk j covers global cols 512j .. 512j+EBW[j]
KMAX = 5

F32R = mybir.dt.float32r
F32 = mybir.dt.float32
BF16 = mybir.dt.bfloat16
I32 = mybir.dt.int32
AX = mybir.AxisListType
AFT = mybir.ActivationFunctionType
ALU = mybir.AluOpType

# E-chunks (contraction tiling) and feature chunks
ECH = [(0, 128), (128, 128), (256, 44)]
GCH = [(g * 128, min(128, F - g * 128)) for g in range(8)]
# valid filter-shifts k contributing to feature chunk g
KSET = [range(3), range(3), range(4), range(4), range(5), range(5), range(5), range(5)]
# per-chunk segments (p0, pm, f): feature sub-ranges belonging to one conv unit
SEG = [
    [(0, 128, 3)], [(0, 128, 3)],
    [(0, 44, 3), (44, 84, 4)],
    [(0, 128, 4)],
    [(0, 88, 4), (88, 40, 5)],
    [(0, 128, 5)], [(0, 128, 5)], [(0, 104, 5)],
]
# token position chunks: (start, width, n_rows, out_col0); first two = pred, last = ref
POS = [(0, 512, 4, 0), (512, 512, 4, 4), (1024, 256, 2, 8)]

_CACHE = {}


def _build_program():
    nc = bacc.Bacc("TRN2", target_bir_lowering=False, debug=False, num_devices=N_CORES)

    d_idx = nc.dram_tensor("idx", [L, RPC], I32, kind="ExternalInput")
    d_emb = nc.dram_tensor("emb", [V, E], BF16, kind="ExternalInput")
    d_wp = nc.dram_tensor("wp", [KMAX, E, F], BF16, kind="ExternalInput")
    d_wr = nc.dram_tensor("wr", [KMAX, E, F], BF16, kind="ExternalInput")
    d_hwt = nc.dram_tensor("hwt", [2, F, F], BF16, kind="ExternalInput")
    d_cb = nc.dram_tensor("cb", [F, 2], F32, kind="ExternalInput")
    d_hwb = nc.dram_tensor("hwb", [2, F], F32, kind="ExternalInput")
    d_lint = nc.dram_tensor("lint", [F, NCLS], F32, kind="ExternalInput")
    d_linb = nc.dram_tensor("linb", [1, NCLS], F32, kind="ExternalInput")
    d_ident = nc.dram_tensor("ident", [128, 128], F32, kind="ExternalInput")
    d_identb = nc.dram_tensor("identb", [128, 128], BF16, kind="ExternalInput")
    d_ones = nc.dram_tensor("ones", [1, RPC], F32, kind="ExternalInput")
    d_onesb = nc.dram_tensor("onesb", [1, RPC], BF16, kind="ExternalInput")
    d_hwbb = nc.dram_tensor("hwbb", [1, 2 * F], BF16, kind="ExternalInput")
    d_res = nc.dram_tensor("res", [RPC, NCLS], F32, kind="ExternalOutput")

    with tile.TileContext(nc) as tc:
        _emit(nc, tc, d_idx, d_emb, d_wp, d_wr, d_hwt, d_cb, d_hwb, d_lint,
              d_linb, d_ident, d_identb, d_ones, d_onesb, d_hwbb, d_res)
    nc.finalize()
    return nc


def _emit(nc, tc, d_idx, d_emb, d_wp, d_wr, d_hwt, d_cb, d_hwb, d_lint,
          d_linb, d_ident, d_identb, d_ones, d_onesb, d_hwbb, d_res):
    STAGE = int(os.environ.get("K_STAGE", "99"))
    from contextlib import ExitStack
    ctx = ExitStack()
    singles = ctx.enter_context(tc.tile_pool(name="singles", bufs=1))
    gpool = ctx.enter_context(tc.tile_pool(name="gather", bufs=3))
    convw = ctx.enter_context(tc.tile_pool(name="convw", bufs=54))
    hwtp = ctx.enter_context(tc.tile_pool(name="hwtp", bufs=5))
    hwx = ctx.enter_context(tc.tile_pool(name="hwx", bufs=8))
    small = ctx.enter_context(tc.tile_pool(name="small", bufs=4))
    ps_tp = ctx.enter_context(tc.tile_pool(name="ps_tp", bufs=1, space="PSUM"))
    ps_cv = ctx.enter_context(tc.tile_pool(name="ps_cv", bufs=3, space="PSUM"))
    ps_hw = ctx.enter_context(tc.tile_pool(name="ps_hw", bufs=4, space="PSUM"))

    # --- small constants ---
    idx_sb = singles.tile([L, RPC], I32)
    nc.sync.dma_start(out=idx_sb[:], in_=d_idx[:])
    id_b = singles.tile([128, 128], BF16)
    nc.scalar.dma_start(out=id_b[:], in_=d_identb[:])

    # --- conv weight streaming (column halves: h=0 -> cols 0:512, h=1 -> 512:1000) ---
    HW_ = [(0, 512), (512, 488)]
    wsb = {}  # (branch, k, c, h) -> tile [cw, wh] F32R
    for h, (h0, wh) in enumerate(HW_):
        ks = range(4) if h == 0 else range(5)
        for k in ks:
            for c, (c0, cw) in enumerate(ECH):
                for br, dram in (("p", d_wp), ("r", d_wr)):
                    t = convw.tile([cw, wh], BF16, tag="w", name=f"w{br}{k}{c}{h}")
                    nc.sync.dma_start(
                        out=t[:], in_=dram[k, c0:c0 + cw, h0:h0 + wh])
                    wsb[(br, k, c, h)] = t

    # small late-need constants AFTER the weight queue (their per-partition
    # 8-byte patterns are descriptor-bound and would delay the weights)
    id_f = singles.tile([128, 128], F32)
    nc.sync.dma_start(out=id_f[:], in_=d_ident[:])
    cb_sb = singles.tile([128, 8, 2], F32)   # [p, g, {pred,ref}] conv biases
    for g, (g0, mg) in enumerate(GCH):
        nc.sync.dma_start(out=cb_sb[:mg, g, :], in_=d_cb[g0:g0 + mg, :])
    lint_sb = singles.tile([128, 8, NCLS], F32R)
    for g, (g0, mg) in enumerate(GCH):
        nc.sync.dma_start(out=lint_sb[:mg, g, :], in_=d_lint[g0:g0 + mg, :].bitcast(F32R))
    linb_sb = singles.tile([1, NCLS], F32R)
    nc.sync.dma_start(out=linb_sb[:], in_=d_linb[:].bitcast(F32R))
    ones_sb = singles.tile([1, RPC], F32R)
    nc.sync.dma_start(out=ones_sb[:], in_=d_ones[:].bitcast(F32R))
    hwbb_sb = singles.tile([1, 2 * F], BF16)
    nc.sync.dma_start(out=hwbb_sb[:], in_=d_hwbb[:])
    onesb_sb = singles.tile([1, RPC], BF16)
    nc.sync.dma_start(out=onesb_sb[:], in_=d_onesb[:])

    # --- PE warm-up: ~3.5us of dummy matmuls while gathers run, so the HAM
    # clock gate reaches 2.4 GHz before real work (inputs: id_b + first
    # weight tile, both early in the DMA queue; output is scratch).
    warm_ps = ps_cv.tile([128, 128], F32, tag="cv", name="warm")
    for _ in range(26):
        nc.tensor.matmul(out=warm_ps[:, :128], lhsT=id_b[:],
                         rhs=id_b[:], start=True, stop=True)

    # --- embedding gather + transpose to channel-major, per pos-chunk tile ---
    embT = [[singles.tile([128, EBW[j]], BF16, tag=f"embT{c}_{j}", name=f"embT{c}_{j}")
             for j in range(3)] for c in range(3)]
    g_ts = [gpool.tile([L, E], BF16, tag=f"emb_g{r}", name=f"emb_g{r}") for r in range(RPC)]
    for r in range(RPC):
        nc.gpsimd.indirect_dma_start(
            out=g_ts[r][:], out_offset=None,
            in_=d_emb[:],
            in_offset=bass.IndirectOffsetOnAxis(ap=idx_sb[:, r:r + 1], axis=0),
        )
    for r in range(RPC):
        j, lc = divmod(r * L, 512)   # destination chunk and chunk-local col
        for c, (c0, cw) in enumerate(ECH):
            tp = ps_tp.tile([128, 128], BF16, tag="tp")
            nc.tensor.transpose(out=tp[:cw, :L], in_=g_ts[r][:, c0:c0 + cw], identity=id_b[:])
            nc.vector.tensor_copy(out=embT[c][j][:cw, lc:lc + L], in_=tp[:cw, :L])
            if lc == 0 and j > 0:
                # first 4 cols also duplicate into the previous chunk's overlap
                nc.vector.tensor_copy(out=embT[c][j - 1][:cw, 512:516], in_=tp[:cw, :4])
    # ref-chunk shift padding (cols 256..384 of chunk 2): any valid data works
    # since those products only land in pooled-out garbage cells
    for c, (c0, cw) in enumerate(ECH):
        nc.vector.tensor_copy(out=embT[c][2][:cw, 256:384], in_=embT[c][0][:cw, 0:128])

    if STAGE <= 1:
        dbg = small.tile([RPC, NCLS], F32, tag="dbg")
        nc.vector.tensor_copy(out=dbg[:], in_=embT[0][0][:RPC, :NCLS])
        nc.sync.dma_start(out=d_res[:], in_=dbg[:])
        ctx.close()
        return

    # --- conv + pool + bias/relu, with pT + pred-highway matmuls interleaved ---
    # hwt tiles are emitted first so the scalar-queue DMAs stream during conv
    hwt_sb = {}
    for b in range(2):
        for g, (g0, mg) in enumerate(GCH):
            t = hwtp.tile([128, F], BF16, tag="hwt", name=f"hwt{b}_{g}")
            nc.scalar.dma_start(out=t[:mg, :], in_=d_hwt[b, g0:g0 + mg, :])
            hwt_sb[(b, g)] = t
    pT = singles.tile([RPC, F], F32)
    hps = {}  # (b, half-index) -> open PSUM accumulator [10, nw]
    NHW = ((0, 512), (512, 488))
    for b in range(2):
        for nh in range(2):
            hps[(b, nh)] = ps_hw.tile([RPC, 512], F32, tag="hp", name=f"hp{b}_{nh}")

    pooledr = [singles.tile([128, RPC], BF16, tag=f"pool{g}", name=f"pool{g}") for g in range(8)]
    pool_g = [singles.tile([128, RPC], F32, tag=f"poolg{g}", name=f"poolg{g}") for g in range(8)]

    def pool_chunk(g, j, psrc):
        # max over valid positions, straight from PSUM. PSUM reads must start
        # at an aligned partition, so mixed-filter chunks do a full-chunk
        # reduce with the larger filter's (smaller) count, then a single-
        # column max fixup for the smaller filter's rows (base partition 0).
        g0, mg = GCH[g]
        nr, oc = POS[j][2], POS[j][3]
        f_max = max(f for (_, _, f) in SEG[g])
        cnt = L - f_max + 1
        src = psrc[0:mg, :].rearrange("p (r t) -> p r t", r=nr)
        nc.vector.reduce_max(
            out=pool_g[g][0:mg, oc:oc + nr], in_=src[:, :, 0:cnt], axis=AX.X)
        for (p0, pm, f) in SEG[g]:
            if f == f_max:
                continue
            for t in range(cnt, L - f + 1):
                nc.vector.tensor_tensor(
                    out=pool_g[g][0:pm, oc:oc + nr],
                    in0=pool_g[g][0:pm, oc:oc + nr],
                    in1=src[0:pm, :, t], op=ALU.max)

    for h in range(2):
        h0 = HW_[h][0]
        for g in range(4 * h, 4 * h + 4):
            g0, mg = GCH[g]
            lo = g0 - h0
            kcs = [(k, c) for k in KSET[g] for c in range(3)]
            psu = {j: ps_cv.tile([128, POS[j][1]], F32, tag="cv", name=f"cv{g}_{j}")
                   for j in (0, 1, 2)}
            for i, (k, c) in enumerate(kcs):
                st, sp = (i == 0), (i == len(kcs) - 1)
                cw = ECH[c][1]
                wpt = wsb[("p", k, c, h)]
                for j in (0, 1):
                    nc.tensor.matmul(
                        out=psu[j][:mg, :512], lhsT=wpt[:cw, lo:lo + mg],
                        rhs=embT[c][j][:cw, k:k + 512], start=st, stop=sp)
            for i, (k, c) in enumerate(kcs):
                cw = ECH[c][1]
                nc.tensor.matmul(
                    out=psu[2][:mg, :256], lhsT=wsb[("r", k, c, h)][:cw, lo:lo + mg],
                    rhs=embT[c][2][:cw, k:k + 256],
                    start=(i == 0), stop=(i == len(kcs) - 1))
            for j in (0, 1, 2):
                pool_chunk(g, j, psu[j])
            nc.scalar.activation(out=pooledr[g][:mg, 0:8], in_=pool_g[g][:mg, 0:8],
                                 func=AFT.Relu, bias=cb_sb[:mg, g, 0:1], scale=1.0)
            nc.scalar.activation(out=pooledr[g][:mg, 8:RPC], in_=pool_g[g][:mg, 8:RPC],
                                 func=AFT.Relu, bias=cb_sb[:mg, g, 1:2], scale=1.0)
            tp = ps_tp.tile([128, 128], BF16, tag="tp")
            nc.tensor.transpose(out=tp[:RPC, :mg], in_=pooledr[g][:mg, :RPC],
                                identity=id_b[:mg, :mg])
            nc.vector.tensor_copy(out=pT[:, g0:g0 + mg], in_=tp[:RPC, :mg])
            for b in range(2):
                for nh, (n0, nw) in enumerate(NHW):
                    nc.tensor.matmul(
                        out=hps[(b, nh)][:RPC, :nw], lhsT=pooledr[g][:mg, :RPC],
                        rhs=hwt_sb[(b, g)][:mg, n0:n0 + nw],
                        start=(g == 0), stop=False)

    if STAGE <= 2:
        dbg = small.tile([RPC, NCLS], F32, tag="dbg")
        nc.vector.tensor_copy(out=dbg[:], in_=pooledr[0][:RPC, :NCLS])
        nc.sync.dma_start(out=d_res[:], in_=dbg[:])
        ctx.close()
        return

    # --- highway epilogue: bias + sigmoid/relu mix per (branch, half) ---
    # Engine partition accesses must start at an aligned base, so both
    # branches compute all 10 rows (the unused rows are free — DVE/ACT cost is
    # free-size-bound) and the pred/ref split happens in the FREE dim after
    # the transpose back to feature-major.
    ho_b = [singles.tile([RPC, F], F32, tag=f"ho{b}", name=f"ho{b}") for b in range(2)]
    # hoT: feature-major highway output; cols 0..7 from pred (b=0), 8..9 from
    # ref (b=1). Transposes + lin matmuls for each column-half run as soon as
    # that half's epilogue chains finish, overlapping the other half.
    hoT = [small.tile([128, RPC], F32R, tag=f"hoT{g % 2}", name=f"hoT{g}") for g in range(8)]
    lps = ps_hw.tile([RPC, 512], F32, tag="hp", name="lps")
    for nh, (n0, nw) in enumerate(NHW):
        for b in range(2):
            hp = hps[(b, nh)]
            # bias via a ones-row matmul (K=1): h += 1 . hw_b[n0:n0+nw]
            nc.tensor.matmul(out=hp[:RPC, :nw], lhsT=onesb_sb[:1, :RPC],
                             rhs=hwbb_sb[:1, b * F + n0:b * F + n0 + nw],
                             start=False, stop=True)
            s = hwx.tile([RPC, 512], F32, tag="s")
            nc.scalar.activation(out=s[:RPC, :nw], in_=hp[:RPC, :nw], func=AFT.Sigmoid)
            rl = hwx.tile([RPC, 512], F32, tag="rl")
            nc.scalar.activation(out=rl[:RPC, :nw], in_=hp[:RPC, :nw], func=AFT.Relu)
            # ho = s*(relu(h) - p) + p
            nc.vector.tensor_tensor(out=rl[:RPC, :nw], in0=rl[:RPC, :nw],
                                    in1=pT[:RPC, n0:n0 + nw], op=ALU.subtract)
            nc.vector.tensor_tensor(out=rl[:RPC, :nw], in0=s[:RPC, :nw],
                                    in1=rl[:RPC, :nw], op=ALU.mult)
            nc.vector.tensor_tensor(out=ho_b[b][:RPC, n0:n0 + nw], in0=rl[:RPC, :nw],
                                    in1=pT[:RPC, n0:n0 + nw], op=ALU.add)
        for g in range(4 * nh, 4 * nh + 4):
            g0, mg = GCH[g]
            for b, (c0_, c1_) in ((0, (0, 8)), (1, (8, RPC))):
                tp2 = ps_tp.tile([128, 128], F32, tag="tp")
                nc.tensor.transpose(out=tp2[:mg, :RPC], in_=ho_b[b][:, g0:g0 + mg],
                                    identity=id_f[:RPC, :RPC])
                nc.vector.tensor_copy(out=hoT[g][:mg, c0_:c1_], in_=tp2[:mg, c0_:c1_])
            nc.tensor.matmul(out=lps[:RPC, :NCLS], lhsT=hoT[g][:mg, :RPC],
                             rhs=lint_sb[:mg, g, :], start=(g == 0), stop=False)
    nc.tensor.matmul(out=lps[:RPC, :NCLS], lhsT=ones_sb[:1, :RPC],
                     rhs=linb_sb[:1, :], start=False, stop=True)

    # --- log_softmax + L2 row normalize ---
    def _dbg_out(ap):
        dbg = small.tile([RPC, NCLS], F32, tag="dbg")
        nc.vector.tensor_copy(out=dbg[:], in_=ap)
        nc.sync.dma_start(out=d_res[:], in_=dbg[:])
        ctx.close()

    mx = small.tile([RPC, 1], F32, tag="mx")
    nc.vector.reduce_max(out=mx[:], in_=lps[:RPC, :NCLS], axis=AX.X)
    t_ = small.tile([RPC, NCLS], F32, tag="t_")
    nc.vector.tensor_scalar(out=t_[:], in0=lps[:RPC, :NCLS], scalar1=mx[:],
                            scalar2=None, op0=ALU.subtract)
    e_ = small.tile([RPC, NCLS], F32, tag="e_")
    se = small.tile([RPC, 1], F32, tag="se")
    nc.scalar.activation(out=e_[:], in_=t_[:], func=AFT.Exp, accum_out=se[:])
    ls = small.tile([RPC, 1], F32, tag="ls")
    nc.scalar.activation(out=ls[:], in_=se[:], func=AFT.Ln)
    pred = small.tile([RPC, NCLS], F32, tag="pred")
    nc.vector.tensor_scalar(out=pred[:], in0=t_[:], scalar1=ls[:],
                            scalar2=None, op0=ALU.subtract)
    # row L2 norm; the reference's max(norm, 1e-8) clamp is a no-op here —
    # a 2-class log-softmax row always has norm >= ln(2)/sqrt(2) ~ 0.49
    sq = small.tile([RPC, NCLS], F32, tag="sq")
    nc.vector.tensor_tensor(out=sq[:], in0=pred[:], in1=pred[:], op=ALU.mult)
    n2 = small.tile([RPC, 1], F32, tag="n2")
    nc.vector.reduce_sum(out=n2[:], in_=sq[:], axis=AX.X)
    sn = small.tile([RPC, 1], F32, tag="sn")
    nc.scalar.activation(out=sn[:], in_=n2[:], func=AFT.Sqrt)
    inv = small.tile([RPC, 1], F32, tag="inv")
    nc.vector.reciprocal(out=inv[:], in_=sn[:])
    outn = small.tile([RPC, NCLS], F32, tag="outn")
    nc.vector.tensor_scalar_mul(out=outn[:], in0=pred[:], scalar1=inv[:])
    nc.sync.dma_start(out=d_res[:], in_=outn[:])
    ctx.close()


def _pack_inputs(inputs):
    """Host-side packing: per-core index slices + shared packed weight arrays."""
    f32 = np.float32
    x = np.asarray(inputs["x"]).astype(np.int32)                  # [64, 128]
    wp = np.zeros((KMAX, E, F), f32)
    wr = np.zeros((KMAX, E, F), f32)
    offs = [0, 300, 600]
    for ui, (f, n) in enumerate(zip(FS, NF)):
        o = offs[ui]
        cw = np.asarray(inputs[f"conv_w{f}"], f32)                # [f, E, n]
        cwr = np.asarray(inputs[f"convr_w{f}"], f32)
        for k in range(f):
            wp[k, :, o:o + n] = cw[k]
            wr[k, :, o:o + n] = cwr[k]
    cb = np.stack([
        np.concatenate([np.asarray(inputs[f"conv_b{f}"], f32) for f in FS]),
        np.concatenate([np.asarray(inputs[f"convr_b{f}"], f32) for f in FS]),
    ], axis=1)                                                    # [1000, 2]
    hwt = np.stack([np.asarray(inputs["hw_w"], f32).T.copy(),
                    np.asarray(inputs["hwr_w"], f32).T.copy()]
                   ).astype(ml_dtypes.bfloat16)                   # [2, 1000, 1000]
    hwb = np.stack([np.asarray(inputs["hw_b"], f32),
                    np.asarray(inputs["hwr_b"], f32)])            # [2, 1000]
    lint = np.asarray(inputs["lin_w"], f32).T.copy()              # [1000, 2]
    linb = np.asarray(inputs["lin_b"], f32).reshape(1, NCLS)
    emb = np.ascontiguousarray(np.asarray(inputs["emb_w"], f32)).astype(ml_dtypes.bfloat16)
    ident = np.eye(128, dtype=f32)

    shared = dict(emb=emb, wp=wp.astype(ml_dtypes.bfloat16),
                  wr=wr.astype(ml_dtypes.bfloat16), hwt=hwt, cb=cb, hwb=hwb,
                  lint=lint, linb=linb, ident=ident,
                  identb=ident.astype(ml_dtypes.bfloat16),
                  ones=np.ones((1, RPC), f32),
                  onesb=np.ones((1, RPC), ml_dtypes.bfloat16),
                  hwbb=hwb.reshape(1, 2 * F).astype(ml_dtypes.bfloat16))
    in_maps = []
    for c in range(N_CORES):
        rows = list(range(8 * c, 8 * c + 8)) + [2 * c, 2 * c + 1]
        idx = np.ascontiguousarray(x[rows].T)                     # [128, 10]
        in_maps.append(dict(idx=idx, **shared))
    return in_maps


def run_cores(inputs, trace=False, **kw):
    """Compile (cached) and run on 8 cores; returns (per-core results, BassKernelResults)."""
    if "nc" not in _CACHE:
        _CACHE["nc"] = _build_program()
    nc = _CACHE["nc"]
    in_maps = _pack_inputs(inputs)
    res = run_bass_kernel_spmd(nc, in_maps, list(range(N_CORES)), trace=trace, **kw)
    return res.results, res


def kernel(**inputs) -> np.ndarray:
    results, _ = run_cores(inputs)
    pn = np.concatenate([results[c]["res"][0:8] for c in range(N_CORES)])   # [64, 2]
    rn = np.concatenate([results[c]["res"][8:RPC] for c in range(N_CORES)])  # [16, 2]
    # values[i] = sum_j cos(rn_j, pn_i) = pn_i . sum_j rn_j ; out = log(values/sum)
    s = rn.sum(axis=0)
    values = pn @ s
    return np.log(values / values.sum()).astype(np.float32)



# revision 14
# speedup vs baseline: 1.3191x; 1.3191x over previous
"""Trainium2 Bass kernel for nn_Discriminator (conv-highway discriminator + cosine retrieval).

Model (per reference):
  emb = emb_w[x]                          # [64, 128, 300]
  pred     = branch(emb, conv_w*, hw_w)   # [64, 2] log-softmax
  pred_ref = branch(emb, convr_*, hwr_w)  # only rows 0..15 are used
  values[i] = sum_j cos(pred_ref[j], pred[i]);  out = log(values / values.sum())

Sharding: 80 row-units (64 pred + 16 ref) split 10 per core: core c computes
pred rows 8c..8c+7 and ref rows 2c, 2c+1.  Each core returns RAW logits
[10, 2]; the host does log-softmax + L2-normalize + the tiny cosine-sum and
log normalizer (O(B) work).

On-device pipeline per core (v1, HAM-warm restructure):
  - per-row indirect-DMA gathers ordered ref rows first -> transpose to
    channel-major via REGULAR matmuls against an identity rhs (these count as
    PE activity for the HAM clock gate, unlike transpose-mode)
  - conv-as-matmul (bf16) with the E=300 remainder rows (256:300) shift-packed
    across filter taps into 88/88/44-row K-tiles (88 MMs per position chunk
    instead of 102)
  - phase order follows gather arrival: ref chunk (N=256, needs only gathers
    8,9 + ref weights) right after warm-up, then pred chunk j=0 (gathers 0-3),
    then j=1 (gathers 4-7), so the PE never idles past a HAM window
  - highway (bf16) batch-stationary matmuls interleaved per g in the last
    phase; bias folded as a ones-row matmul
  - epilogue: sigmoid/relu/mix in bf16, transpose back via regular matmuls,
    linear to logits; log-softmax/normalize moved to host
  - single ACT table set (sigmoid_and_others covers sigmoid+relu), preloaded
    by a dummy activation at t=0
fp32 PSUM accumulation throughout; bf16 operands (PE 2.4 GHz HAM clock).
"""

import os
import sys

for _p in ("/opt/trn_rl_repo", "/root/.axon_site/_ro/trn_rl_repo"):
    if os.path.isdir(_p) and _p not in sys.path:
        sys.path.insert(0, _p)

import ml_dtypes
import numpy as np

import concourse.bass as bass
import concourse.mybir as mybir
import concourse.tile as tile
from concourse import bacc
from concourse.bass_utils import run_bass_kernel_spmd

# ---- problem constants (hardcoded per spec) ----
B, REF, L, V, E = 64, 16, 128, 50000, 300
FS = [3, 4, 5]
NF = [300, 300, 400]
F = 1000                      # sum(NF)
NCLS = 2
N_CORES = 8
RPC = 10                      # rows per core: 8 pred + 2 ref
KMAX = 5

F32 = mybir.dt.float32
BF16 = mybir.dt.bfloat16
I32 = mybir.dt.int32
AX = mybir.AxisListType
AFT = mybir.ActivationFunctionType
ALU = mybir.AluOpType

# full 128-row contraction chunks; rows 256:300 go through the shift-packed
# remainder tiles RA (shifts 0,1), RB (shifts 2,3), RC (shift 4)
ECH = [(0, 128), (128, 128)]
GCH = [(g * 128, min(128, F - g * 128)) for g in range(8)]
# valid filter-shifts k contributing to feature chunk g (zero-padded weights
# make over-inclusive k harmless)
KSET = [range(3), range(3), range(4), range(4), range(5), range(5), range(5), range(5)]
# per-chunk segments (p0, pm, f): feature sub-ranges belonging to one conv unit
SEG = [
    [(0, 128, 3)], [(0, 128, 3)],
    [(0, 44, 3), (44, 84, 4)],
    [(0, 128, 4)],
    [(0, 88, 4), (88, 40, 5)],
    [(0, 128, 5)], [(0, 128, 5)], [(0, 104, 5)],
]
# remainder tiles used per g: RA+RB always (zero-padded), RC only when f=5
NREM = [2, 2, 2, 2, 3, 3, 3, 3]
# token position chunks: (width, n_rows, out_col0); j=2 is the ref chunk
POS = [(512, 4, 0), (512, 4, 4), (256, 2, 8)]
EBW = [520, 520, 264]         # embT chunk widths (incl. shift overlap cols)
# conv weight tile layout: [128, 4*512 + 5*488]; h=0 k-tiles then h=1 k-tiles
HW_ = [(0, 512), (512, 488)]
WOFF = {0: lambda k: k * 512, 1: lambda k: 2048 + k * 488}

_CACHE = {}


def _build_program():
    nc = bacc.Bacc("TRN2", target_bir_lowering=False, debug=False, num_devices=N_CORES)

    d_idx = nc.dram_tensor("idx", [L, RPC], I32, kind="ExternalInput")
    d_emb = nc.dram_tensor("emb", [V, E], BF16, kind="ExternalInput")
    d_wc = {(br, c): nc.dram_tensor(f"wc_{br}{c}", [128, 4488], BF16, kind="ExternalInput")
            for br in "pr" for c in range(2)}
    d_rem = {br: nc.dram_tensor(f"rem_{br}", [3, 108, F], BF16, kind="ExternalInput")
             for br in "pr"}
    d_hwt = nc.dram_tensor("hwt", [2, F, F], BF16, kind="ExternalInput")
    d_cb = nc.dram_tensor("cb", [F, 2], F32, kind="ExternalInput")
    d_lint = nc.dram_tensor("lint", [F, NCLS], BF16, kind="ExternalInput")
    d_linb = nc.dram_tensor("linb", [1, NCLS], BF16, kind="ExternalInput")
    d_identb = nc.dram_tensor("identb", [128, 128], BF16, kind="ExternalInput")
    d_onesb = nc.dram_tensor("onesb", [1, RPC], BF16, kind="ExternalInput")
    d_hwbb = nc.dram_tensor("hwbb", [1, 2 * F], BF16, kind="ExternalInput")
    d_res = nc.dram_tensor("res", [RPC, NCLS], F32, kind="ExternalOutput")

    with tile.TileContext(nc) as tc:
        _emit(nc, tc, d_idx, d_emb, d_wc, d_rem, d_hwt, d_cb, d_lint, d_linb,
              d_identb, d_onesb, d_hwbb, d_res)
    nc.finalize()
    return nc


def _emit(nc, tc, d_idx, d_emb, d_wc, d_rem, d_hwt, d_cb, d_lint, d_linb,
          d_identb, d_onesb, d_hwbb, d_res):
    STAGE = int(os.environ.get("K_STAGE", "99"))
    from contextlib import ExitStack
    ctx = ExitStack()
    singles = ctx.enter_context(tc.tile_pool(name="singles", bufs=1))
    hwtp = ctx.enter_context(tc.tile_pool(name="hwtp", bufs=1))
    hwx = ctx.enter_context(tc.tile_pool(name="hwx", bufs=8))
    small = ctx.enter_context(tc.tile_pool(name="small", bufs=4))
    ps_tp = ctx.enter_context(tc.tile_pool(name="ps_tp", bufs=1, space="PSUM"))
    ps_cv = ctx.enter_context(tc.tile_pool(name="ps_cv", bufs=3, space="PSUM"))
    ps_hw = ctx.enter_context(tc.tile_pool(name="ps_hw", bufs=4, space="PSUM"))

    # --- ACT table preload: sigmoid_and_others covers sigmoid+relu+copy.
    # Emitting a sigmoid FIRST pins that set so no later table switch occurs.
    scrap = singles.tile([1, 2], F32)
    nc.vector.memset(scrap[:], 0.0)
    scrap2 = singles.tile([1, 2], F32)
    nc.scalar.activation(out=scrap2[:], in_=scrap[:], func=AFT.Sigmoid)

    # --- small constants ---
    idx_sb = singles.tile([L, RPC], I32)
    nc.sync.dma_start(out=idx_sb[:], in_=d_idx[:])
    id_b = singles.tile([128, 128], BF16)
    nc.sync.dma_start(out=id_b[:], in_=d_identb[:])

    # --- PE warm-up: dummy matmuls (junk values, scratch psum) to trip the
    # HAM clock gate while the gathers + ref weights stream in.
    junk = singles.tile([128, 512], BF16)
    nc.vector.memset(junk[:], 0.0)
    for w_ in range(12):
        warm_ps = ps_cv.tile([128, 512], F32, tag="cv", name=f"warm{w_ % 3}")
        nc.tensor.matmul(out=warm_ps[:], lhsT=id_b[:], rhs=junk[:],
                         start=True, stop=True)

    # --- gathers: one indirect DMA per row-unit, ref rows (8, 9) FIRST so
    # the ref conv phase can start right after warm-up
    G = {}
    for r in [8, 9] + list(range(8)):
        t = singles.tile([L, E], BF16, tag=f"emb_g{r}", name=f"emb_g{r}")
        nc.gpsimd.indirect_dma_start(
            out=t[:], out_offset=None, in_=d_emb[:],
            in_offset=bass.IndirectOffsetOnAxis(ap=idx_sb[:, r:r + 1], axis=0))
        G[r] = t

    # --- conv weights, ref branch first, each split h0/h1 for fine deps ---
    wsb = {}   # (br, c) -> [128, 4488]
    rem_sb = {}  # (br, t) -> [88, F]
    for br in "rp":
        for c in range(2):
            wsb[(br, c)] = hwtp.tile([128, 4488], BF16, tag=f"wc{br}{c}",
                                     name=f"wc{br}{c}")
        for h0, h1 in ((0, 2048), (2048, 4488)):
            for c in range(2):
                nc.sync.dma_start(out=wsb[(br, c)][:, h0:h1],
                                  in_=d_wc[(br, c)][:, h0:h1])
        for ti in range(3):
            rt = hwtp.tile([108, F], BF16, tag=f"rem{br}{ti}", name=f"rem{br}{ti}")
            nc.sync.dma_start(out=rt[:], in_=d_rem[br][ti, :, :])
            rem_sb[(br, ti)] = rt

    # small consts on the scalar queue (parallel to the big sync stream)
    cb_sb = singles.tile([128, 8, 2], F32)
    for g, (g0, mg) in enumerate(GCH):
        nc.scalar.dma_start(out=cb_sb[:mg, g, :], in_=d_cb[g0:g0 + mg, :])
    lint_sb = singles.tile([128, 8, NCLS], BF16)
    for g, (g0, mg) in enumerate(GCH):
        nc.scalar.dma_start(out=lint_sb[:mg, g, :], in_=d_lint[g0:g0 + mg, :])
    linb_sb = singles.tile([1, NCLS], BF16)
    nc.scalar.dma_start(out=linb_sb[:], in_=d_linb[:])
    onesb_sb = singles.tile([1, RPC], BF16)
    nc.scalar.dma_start(out=onesb_sb[:], in_=d_onesb[:])
    hwbb_sb = singles.tile([1, 2 * F], BF16)
    nc.scalar.dma_start(out=hwbb_sb[:], in_=d_hwbb[:])

    # highway weights (sync queue, after conv weights; g-major so the per-g
    # highway matmuls unblock progressively)
    hwt_sb = {}
    for g, (g0, mg) in enumerate(GCH):
        for b in range(2):
            t = hwtp.tile([128, F], BF16, tag=f"hwt{b}{g}", name=f"hwt{b}_{g}")
            nc.sync.dma_start(out=t[:mg, :], in_=d_hwt[b, g0:g0 + mg, :])
            hwt_sb[(b, g)] = t

    # --- embT/embR: channel-major embeddings ---
    # embT[c][j]: [128, EBW[j]] for the two full E-chunks
    # embR[t][j]: shift-packed remainder tiles (RA/RB: 88 rows, RC: 44)
    embT = [[singles.tile([128, EBW[j]], BF16, tag=f"embT{c}_{j}", name=f"embT{c}_{j}")
             for j in range(3)] for c in range(2)]
    emb2 = [singles.tile([44, EBW[j]], BF16, tag=f"emb2_{j}", name=f"emb2_{j}")
            for j in range(3)]
    # shift-packed remainder tiles: shift 2t at partitions 0:44, shift 2t+1 at
    # partitions 64:108 (engine writes need 32-aligned partition bases); rows
    # 44:64 are memset (weights there are zero, but NaN garbage would poison
    # the accumulation)
    embR = [[singles.tile([108, EBW[j]], BF16, tag=f"embR{t}_{j}", name=f"embR{t}_{j}")
             for j in range(3)] for t in range(3)]

    def transpose_rows(rows):
        # gather-tile [128 tok, 300] slices -> channel-major via regular matmul
        for r in rows:
            src = G[r]
            j, lc = divmod(r * L, 512)
            for c, (c0, cw) in enumerate(ECH):
                tp = ps_cv.tile([128, 512], F32, tag="cv", name=f"tpg{r}_{c}")
                nc.tensor.matmul(out=tp[:cw, :L], lhsT=src[:, c0:c0 + cw],
                                 rhs=id_b[:], start=True, stop=True)
                nc.vector.tensor_copy(out=embT[c][j][:cw, lc:lc + L], in_=tp[:cw, :L])
            tp = ps_cv.tile([128, 512], F32, tag="cv", name=f"tpg{r}_2")
            nc.tensor.matmul(out=tp[:44, :L], lhsT=src[:, 256:300],
                             rhs=id_b[:], start=True, stop=True)
            nc.vector.tensor_copy(out=emb2[j][:44, lc:lc + L], in_=tp[:44, :L])

    def fill_and_shift(j):
        # fill the shift-overlap columns with valid (don't-care) data, then
        # build the shift-packed remainder tiles for this position chunk
        w = POS[j][0]
        for c in range(2):
            nc.vector.tensor_copy(out=embT[c][j][:, w:w + 8], in_=embT[c][j][:, 0:8])
        nc.vector.tensor_copy(out=emb2[j][:44, w:w + 8], in_=emb2[j][:44, 0:8])
        for t in range(3):
            nc.vector.memset(embR[t][j][:, :], 0.0)
            for half in range(2):
                sh = 2 * t + half
                if sh > 4:
                    continue
                nc.vector.tensor_copy(out=embR[t][j][64 * half:64 * half + 44, 0:w + 4 - sh],
                                      in_=emb2[j][:44, sh:w + 4])

    pool_g = [singles.tile([128, RPC], F32, tag=f"poolg{g}", name=f"poolg{g}") for g in range(8)]

    def pool_chunk(g, j, psrc):
        g0, mg = GCH[g]
        w, nr, oc = POS[j]
        f_max = max(f for (_, _, f) in SEG[g])
        cnt = L - f_max + 1
        src = psrc[0:mg, :].rearrange("p (r t) -> p r t", r=nr)
        nc.vector.reduce_max(
            out=pool_g[g][0:mg, oc:oc + nr], in_=src[:, :, 0:cnt], axis=AX.X)
        for (p0, pm, f) in SEG[g]:
            if f == f_max:
                continue
            for t in range(cnt, L - f + 1):
                nc.vector.tensor_tensor(
                    out=pool_g[g][0:pm, oc:oc + nr],
                    in0=pool_g[g][0:pm, oc:oc + nr],
                    in1=src[0:pm, :, t], op=ALU.max)

    def conv_mms(g, br, outs):
        # outs: list of (psum_ap, j) accumulated over all K-tiles for chunk g
        g0, mg = GCH[g]
        h = g // 4
        lo = g0 - HW_[h][0]
        steps = []
        for k in KSET[g]:
            for c in range(2):
                steps.append((wsb[(br, c)][:, WOFF[h](k) + lo:WOFF[h](k) + lo + mg],
                              [embT[c][j][:, k:k + POS[j][0]] for j in range(3)]))
        for t in range(NREM[g]):
            kr = 108 if t < 2 else 44
            steps.append((rem_sb[(br, t)][:kr, g0:g0 + mg],
                          [embR[t][j][:kr, 0:POS[j][0]] for j in range(3)]))
        for i, (wap, rhs3) in enumerate(steps):
            st, sp = (i == 0), (i == len(steps) - 1)
            for ps, j in outs:
                nc.tensor.matmul(out=ps, lhsT=wap, rhs=rhs3[j], start=st, stop=sp)

    # ================= phase R: ref chunk (j=2) =================
    transpose_rows([8, 9])
    fill_and_shift(2)
    for g, (g0, mg) in enumerate(GCH):
        psu2 = ps_cv.tile([128, 512], F32, tag="cv", name=f"cvR{g}")
        conv_mms(g, "r", [(psu2[:mg, :256], 2)])
        pool_chunk(g, 2, psu2[:, :256])

    if STAGE <= 1:
        dbg = small.tile([RPC, NCLS], F32, tag="dbg")
        nc.vector.tensor_copy(out=dbg[:], in_=pool_g[0][:RPC, :NCLS])
        nc.sync.dma_start(out=d_res[:], in_=dbg[:])
        ctx.close()
        return

    # ================= phase P0: pred chunk j=0 (gathers 0-3) =================
    transpose_rows(range(4))
    fill_and_shift(0)
    for g, (g0, mg) in enumerate(GCH):
        psu0 = ps_cv.tile([128, 512], F32, tag="cv", name=f"cvA{g}")
        conv_mms(g, "p", [(psu0[:mg, :], 0)])
        pool_chunk(g, 0, psu0)

    # ================= phase P1: pred chunk j=1 (gathers 4-7) =================
    transpose_rows(range(4, 8))
    fill_and_shift(1)

    pooledr = [singles.tile([128, RPC], BF16, tag=f"pool{g}", name=f"pool{g}") for g in range(8)]
    pT = singles.tile([RPC, F], BF16)
    hps = {}
    NHW = ((0, 512), (512, 488))
    for b in range(2):
        for nh in range(2):
            hps[(b, nh)] = ps_hw.tile([RPC, 512], F32, tag="hp", name=f"hp{b}_{nh}")

    for g, (g0, mg) in enumerate(GCH):
        psu1 = ps_cv.tile([128, 512], F32, tag="cv", name=f"cvB{g}")
        conv_mms(g, "p", [(psu1[:mg, :], 1)])
        pool_chunk(g, 1, psu1)
        nc.scalar.activation(out=pooledr[g][:mg, 0:8], in_=pool_g[g][:mg, 0:8],
                             func=AFT.Relu, bias=cb_sb[:mg, g, 0:1], scale=1.0)
        nc.scalar.activation(out=pooledr[g][:mg, 8:RPC], in_=pool_g[g][:mg, 8:RPC],
                             func=AFT.Relu, bias=cb_sb[:mg, g, 1:2], scale=1.0)
        # pooled row-major copy for the highway mix epilogue
        tp = ps_tp.tile([128, 128], F32, tag="tp")
        nc.tensor.matmul(out=tp[:RPC, :mg], lhsT=pooledr[g][:mg, :RPC],
                         rhs=id_b[:mg, :mg], start=True, stop=True)
        nc.vector.tensor_copy(out=pT[:, g0:g0 + mg], in_=tp[:RPC, :mg])
        for b in range(2):
            for nh, (n0, nw) in enumerate(NHW):
                nc.tensor.matmul(
                    out=hps[(b, nh)][:RPC, :nw], lhsT=pooledr[g][:mg, :RPC],
                    rhs=hwt_sb[(b, g)][:mg, n0:n0 + nw],
                    start=(g == 0), stop=False)

    if STAGE <= 2:
        dbg = small.tile([RPC, NCLS], F32, tag="dbg")
        nc.vector.tensor_copy(out=dbg[:], in_=pooledr[0][:RPC, :NCLS])
        nc.sync.dma_start(out=d_res[:], in_=dbg[:])
        ctx.close()
        return

    # ================= epilogue: highway mix + linear =================
    ho_b = [singles.tile([RPC, F], BF16, tag=f"ho{b}", name=f"ho{b}") for b in range(2)]
    hoT = [small.tile([128, RPC], BF16, tag=f"hoT{g % 2}", name=f"hoT{g}") for g in range(8)]
    lps = ps_hw.tile([RPC, 512], F32, tag="hp", name="lps")
    for nh, (n0, nw) in enumerate(NHW):
        for b in range(2):
            hp = hps[(b, nh)]
            # bias via a ones-row matmul (K=1): h += 1 . hw_b[n0:n0+nw]
            nc.tensor.matmul(out=hp[:RPC, :nw], lhsT=onesb_sb[:1, :RPC],
                             rhs=hwbb_sb[:1, b * F + n0:b * F + n0 + nw],
                             start=False, stop=True)
            s = hwx.tile([RPC, 512], BF16, tag="s")
            nc.scalar.activation(out=s[:RPC, :nw], in_=hp[:RPC, :nw], func=AFT.Sigmoid)
            rl = hwx.tile([RPC, 512], BF16, tag="rl")
            nc.scalar.activation(out=rl[:RPC, :nw], in_=hp[:RPC, :nw], func=AFT.Relu)
            # ho = s*(relu(h) - p) + p
            nc.vector.tensor_tensor(out=rl[:RPC, :nw], in0=rl[:RPC, :nw],
                                    in1=pT[:RPC, n0:n0 + nw], op=ALU.subtract)
            nc.vector.tensor_tensor(out=rl[:RPC, :nw], in0=s[:RPC, :nw],
                                    in1=rl[:RPC, :nw], op=ALU.mult)
            nc.vector.tensor_tensor(out=ho_b[b][:RPC, n0:n0 + nw], in0=rl[:RPC, :nw],
                                    in1=pT[:RPC, n0:n0 + nw], op=ALU.add)
        for g in range(4 * nh, 4 * nh + 4):
            g0, mg = GCH[g]
            for b, (c0_, c1_) in ((0, (0, 8)), (1, (8, RPC))):
                tp2 = ps_cv.tile([128, 512], F32, tag="cv", name=f"tpho{g}_{b}")
                nc.tensor.matmul(out=tp2[:mg, :RPC], lhsT=ho_b[b][:, g0:g0 + mg],
                                 rhs=id_b[:RPC, :RPC], start=True, stop=True)
                nc.scalar.activation(out=hoT[g][:mg, c0_:c1_], in_=tp2[:mg, c0_:c1_],
                                     func=AFT.Copy)
            nc.tensor.matmul(out=lps[:RPC, :NCLS], lhsT=hoT[g][:mg, :RPC],
                             rhs=lint_sb[:mg, g, :], start=(g == 0), stop=False)
    nc.tensor.matmul(out=lps[:RPC, :NCLS], lhsT=onesb_sb[:1, :RPC],
                     rhs=linb_sb[:1, :], start=False, stop=True)

    outn = small.tile([RPC, NCLS], F32, tag="outn")
    nc.vector.tensor_copy(out=outn[:], in_=lps[:RPC, :NCLS])
    nc.sync.dma_start(out=d_res[:], in_=outn[:])
    ctx.close()


def _pack_inputs(inputs):
    """Host-side packing: per-core index slices + shared packed weight arrays."""
    f32 = np.float32
    bf16 = ml_dtypes.bfloat16
    x = np.asarray(inputs["x"]).astype(np.int32)                  # [64, 128]
    wfull = {"p": np.zeros((KMAX, E, F), f32), "r": np.zeros((KMAX, E, F), f32)}
    offs = [0, 300, 600]
    for ui, (f, n) in enumerate(zip(FS, NF)):
        o = offs[ui]
        cw = np.asarray(inputs[f"conv_w{f}"], f32)                # [f, E, n]
        cwr = np.asarray(inputs[f"convr_w{f}"], f32)
        for k in range(f):
            wfull["p"][k, :, o:o + n] = cw[k]
            wfull["r"][k, :, o:o + n] = cwr[k]
    shared = {}
    for br in "pr":
        w = wfull[br]
        for c, (c0, cwd) in enumerate(ECH):
            t = np.zeros((128, 4488), f32)
            for h, (h0, wh) in enumerate(HW_):
                ks = 4 if h == 0 else 5
                for k in range(ks):
                    t[:cwd, WOFF[h](k):WOFF[h](k) + wh] = w[k, c0:c0 + cwd, h0:h0 + wh]
            shared[f"wc_{br}{c}"] = t.astype(bf16)
        rem = np.zeros((3, 108, F), f32)
        for t_ in range(3):
            for half in range(2):
                sh = 2 * t_ + half
                if sh > 4:
                    continue
                rem[t_, 64 * half:64 * half + 44, :] = w[sh, 256:300, :]
        shared[f"rem_{br}"] = rem.astype(bf16)
    cb = np.stack([
        np.concatenate([np.asarray(inputs[f"conv_b{f}"], f32) for f in FS]),
        np.concatenate([np.asarray(inputs[f"convr_b{f}"], f32) for f in FS]),
    ], axis=1)                                                    # [1000, 2]
    hwt = np.stack([np.asarray(inputs["hw_w"], f32).T.copy(),
                    np.asarray(inputs["hwr_w"], f32).T.copy()]).astype(bf16)
    hwb = np.stack([np.asarray(inputs["hw_b"], f32),
                    np.asarray(inputs["hwr_b"], f32)])            # [2, 1000]
    shared.update(
        emb=np.ascontiguousarray(np.asarray(inputs["emb_w"], f32)).astype(bf16),
        hwt=hwt, cb=cb,
        lint=np.asarray(inputs["lin_w"], f32).T.copy().astype(bf16),
        linb=np.asarray(inputs["lin_b"], f32).reshape(1, NCLS).astype(bf16),
        identb=np.eye(128, dtype=f32).astype(bf16),
        onesb=np.ones((1, RPC), bf16),
        hwbb=hwb.reshape(1, 2 * F).astype(bf16))
    in_maps = []
    for c in range(N_CORES):
        rows = list(range(8 * c, 8 * c + 8)) + [2 * c, 2 * c + 1]
        idx = np.ascontiguousarray(x[rows].T)                     # [128, 10]
        in_maps.append(dict(idx=idx, **shared))
    return in_maps


def run_cores(inputs, trace=False, **kw):
    """Compile (cached) and run on 8 cores; returns (per-core results, BassKernelResults)."""
    if "nc" not in _CACHE:
        _CACHE["nc"] = _build_program()
    nc = _CACHE["nc"]
    in_maps = _pack_inputs(inputs)
    res = run_bass_kernel_spmd(nc, in_maps, list(range(N_CORES)), trace=trace, **kw)
    return res.results, res


def combine(results) -> np.ndarray:
    """Host epilogue: per-row log-softmax + L2-normalize, then the cosine sum."""
    logits = np.concatenate([results[c]["res"] for c in range(N_CORES)]).astype(np.float64)
    m = logits.max(axis=1, keepdims=True)
    ls = m + np.log(np.exp(logits - m).sum(axis=1, keepdims=True))
    pred = logits - ls                                            # [80, 2]
    n = np.maximum(np.linalg.norm(pred, axis=1, keepdims=True), 1e-8)
    pn = pred / n
    is_pred = np.tile([True] * 8 + [False] * 2, N_CORES)
    p, r = pn[is_pred], pn[~is_pred]
    values = p @ r.sum(axis=0)
    return np.log(values / values.sum()).astype(np.float32)


def kernel(**inputs) -> np.ndarray:
    results, _ = run_cores(inputs)
    return combine(results)
